# revision 10
# baseline (speedup 1.0000x reference)
"""DistanceNetwork (retrieval kNN cosine similarity) TRN2 Bass kernel.

reference:
    input_mag = rsqrt(max(sum(input**2), eps))              # global scalar
    support_mag = rsqrt(max(sum(support**2, axis=1), eps))  # [n]
    out[n, b, 0] = dot(support[n], input[b]) * support_mag[n] * input_mag

Shapes (hardcoded): support_set [8192, 1024] f32, input_image [2048, 1024] f32,
out [8192, 2048, 1] f32.

Sharding: support rows split across 8 cores (1024 rows / core); input_image
replicated.  No collectives.

Both operands are host-converted to bf16 (~2.1e-3 scale-relative error vs the
2e-2 gate): bf16 halves input DMA bytes and the PE streams bf16 matmuls at
~222ns issue cadence per 512-row matmul (~1 row/cycle sustained).

Measured-trace-driven schedule (per core):
  - loads (sync queue): s[kt]/x[kt][bt0] interleaved per kt so the bt0 matmul
    pass streams kt-by-kt behind the DMA; x for bt=1..3 are ONE DMA each
    (per-DMA issue cost ~0.6us on the queue engine paces many-small-DMA
    streams, not bandwidth); finally the support shard AGAIN in row-major
    form (s_r, 2MB) whose per-nt ACT Square+accum gives the support norms
    directly in [128, NT] layout -- this removes the s^2 elementwise chain
    (~19us of DVE), the ones-matmul partition reduce, and the DRAM bounce
    transpose of earlier revisions.
  - PE: bt0 kt-major for kt0..5, then kt6+kt7 per-nt so the 8 stops stagger
    and DVE frees each bank before bt1 reaches it; then bt1/bt2/bt3 nt-major.
    PE has zero non-GEMM work.
  - drains on DVE: bt0/1 unscaled (x1.0), bt2/3 fused with the combined
    magnitude scale (comb resolves ~45us, first fused stop ~41us; the short
    drain lag only delays stores, never a psum bank reuse).
  - second pass for bt0/1: scales on the otherwise-idle Pool engine, stores
    on the ACT queue; bt2/3 stores on the sync queue.
"""

import numpy as np
import ml_dtypes

import concourse.bass as bass
import concourse.bacc as bacc
import concourse.bass_isa as bass_isa
import concourse.tile as tile
import concourse.mybir as mybir
from concourse.bass_utils import run_bass_kernel_spmd

F32 = mybir.dt.float32
BF16 = mybir.dt.bfloat16
AF = mybir.ActivationFunctionType
ALU = mybir.AluOpType

D = 1024          # feature dim (contraction)
NS = 1024         # support rows per core
B = 2048          # query batch (replicated per core)
KT = D // 128     # 8 contraction tiles
NT = NS // 128    # 8 output-partition tiles
BT = B // 512     # 4 moving-dim chunks
EPS = 1e-10
N_CORES = 8


def _newton_rsqrt(nc, pool, a_ap, seed_ap, shape, pfx, iters=2):
    """r ~= rsqrt(a) refined from seed (1/sqrt via LUT) with Newton steps.

    r <- r * (1.5 - 0.5 * a * r * r).  All tiles [P, W] f32.
    """
    r = seed_ap
    for i in range(iters):
        t = pool.tile(shape, F32, tag=f"{pfx}_t{i}", name=f"{pfx}_t{i}")
        nc.vector.tensor_mul(t[:], r, r)
        nc.vector.tensor_mul(t[:], a_ap, t[:])
        nc.vector.tensor_scalar(
            t[:], t[:], -0.5, 1.5, op0=ALU.mult, op1=ALU.add
        )
        r2 = pool.tile(shape, F32, tag=f"{pfx}_r{i}", name=f"{pfx}_r{i}")
        nc.vector.tensor_mul(r2[:], r, t[:])
        r = r2[:]
    return r


def build_nc():
    nc = bacc.Bacc(None, target_bir_lowering=False)
    s_dram = nc.declare_dram_parameter("s_t", [D, NS], BF16, isOutput=False)
    sr_dram = nc.declare_dram_parameter("s_r", [NS, D], BF16, isOutput=False)
    x_dram = nc.declare_dram_parameter("x_t", [D, B], BF16, isOutput=False)
    o_dram = nc.declare_dram_parameter("out", [NS, B], F32, isOutput=True)

    with tile.TileContext(nc) as tc:
        with (
            tc.tile_pool(name="sp", bufs=KT) as sp,
            tc.tile_pool(name="xp", bufs=KT) as xp,
            tc.tile_pool(name="oh", bufs=2 * NT) as oh,      # bt0/1 held
            tc.tile_pool(name="of", bufs=8) as of,           # bt2/3 recycled
            tc.tile_pool(name="s2p", bufs=2) as s2p,
            tc.tile_pool(name="small", bufs=1) as small,
            tc.tile_pool(name="psum", bufs=8, space="PSUM") as psum,
        ):
            # ---- constants ---------------------------------------------------
            ones = small.tile([128, 1], F32)
            nc.vector.memset(ones[:], 1.0)
            # pin ACT's sqrt table set before the Square stream starts, so the
            # mid-kernel Sqrt calls don't force a ~2.7us table reload
            sq_dummy = small.tile([1, 1], F32)
            nc.scalar.activation(sq_dummy[:], ones[0:1, 0:1], AF.Sqrt)

            accs = small.tile([128, KT + 3], F32)
            accs_s = small.tile([128, NT], F32)
            s_sb = [None] * KT
            x0_sb = [None] * KT
            xr_sb = [None] * BT   # bt=1..3: [128, KT, 512]

            # ---- input DMAs (sync queue) ------------------------------------
            for kt in range(KT):
                t = sp.tile([128, NS], BF16, tag="s_sb", name=f"s{kt}")
                nc.sync.dma_start(
                    out=t[:], in_=s_dram[kt * 128:(kt + 1) * 128, :]
                )
                s_sb[kt] = t
                tx = xp.tile([128, 512], BF16, tag="x_sb", name=f"x{kt}_0")
                nc.sync.dma_start(
                    out=tx[:], in_=x_dram[kt * 128:(kt + 1) * 128, 0:512]
                )
                x0_sb[kt] = tx
            for bt in range(1, BT):
                t = xp.tile([128, KT, 512], BF16, tag="xr_sb", name=f"xr{bt}",
                            bufs=3)
                nc.sync.dma_start(
                    out=t[:],
                    in_=x_dram[:, bt * 512:(bt + 1) * 512].rearrange(
                        "(t p) c -> p t c", p=128
                    ),
                )
                xr_sb[bt] = t
            sr_sb = xp.tile([128, NT, D], BF16, tag="sr_sb", name="sr",
                            bufs=1)
            nc.sync.dma_start(
                out=sr_sb[:],
                in_=sr_dram.rearrange("(t p) d -> p t d", p=128),
            )

            def xtile(kt, bt):
                return x0_sb[kt][:] if bt == 0 else xr_sb[bt][:, kt, :]

            # ---- squares on ACT: x^2 per-partition sums into accs columns,
            # s_r^2 per-nt sums = support norms directly in [128, NT] --------
            for kt in range(KT):
                scr = s2p.tile([128, 512], F32, tag="scr", name=f"scr{kt}",
                               bufs=2)
                nc.scalar.activation(
                    scr[:], x0_sb[kt][:], AF.Square,
                    accum_out=accs[:, kt:kt + 1],
                )
            for bt in range(1, BT):
                scrw = s2p.tile([128, KT * 512], F32, tag="scrw",
                                name=f"scrw{bt}", bufs=1)
                nc.scalar.activation(
                    scrw[:], xr_sb[bt][:].rearrange("p t c -> p (t c)"),
                    AF.Square,
                    accum_out=accs[:, KT + bt - 1:KT + bt],
                )
            for nt in range(NT):
                scs = s2p.tile([128, D], F32, tag="scs", name=f"scs{nt}",
                               bufs=2)
                nc.scalar.activation(
                    scs[:], sr_sb[:, nt, :], AF.Square,
                    accum_out=accs_s[:, nt:nt + 1],
                )

            def main_mm(ps_ap, kt, nt, bt):
                nc.tensor.matmul(
                    ps_ap,
                    s_sb[kt][:, nt * 128:(nt + 1) * 128],
                    xtile(kt, bt),
                    start=(kt == 0),
                    stop=(kt == KT - 1),
                )

            # ---- bt=0: kt-major for kt0..5 (streams behind the loads); the
            # last two kt rows run per-nt so the 8 stops stagger ~0.44us and
            # the DVE drains free each bank before bt1 reaches it ------------
            ps_g0 = [
                psum.tile([128, 512], F32, tag="ps", name=f"ps0_{nt}")
                for nt in range(NT)
            ]
            for kt in range(KT - 2):
                for nt in range(NT):
                    main_mm(ps_g0[nt][:], kt, nt, 0)
            for nt in range(NT):
                main_mm(ps_g0[nt][:], KT - 2, nt, 0)
                main_mm(ps_g0[nt][:], KT - 1, nt, 0)

            o_sb = [[None] * NT for _ in range(2)]

            def drain_unscaled(hold_bt, nt, ps_ap):
                o = oh.tile([128, 512], F32, tag="o", name=f"o{hold_bt}_{nt}")
                nc.vector.tensor_scalar(o[:], ps_ap, 1.0, None, op0=ALU.mult)
                o_sb[hold_bt][nt] = o

            for nt in range(NT):
                drain_unscaled(0, nt, ps_g0[nt][:])

            # ---- bt=1: nt-major; magnitude chain sprinkled between drains --
            ps_b1 = [None] * NT
            for nt in range(NT):
                ps_b1[nt] = psum.tile([128, 512], F32, tag="ps",
                                      name=f"ps1_{nt}")
                for kt in range(KT):
                    main_mm(ps_b1[nt][:], kt, nt, 1)
            for nt in range(3):
                drain_unscaled(1, nt, ps_b1[nt][:])
            # s chain (accs_s resolves once the s_r squares land, ~43us)
            smax = small.tile([128, NT], F32)
            nc.vector.tensor_scalar_max(smax[:], accs_s[:], EPS)
            s_sqrt = small.tile([128, NT], F32)
            nc.scalar.activation(s_sqrt[:], smax[:], AF.Sqrt)
            drain_unscaled(1, 3, ps_b1[3][:])
            # x chain
            xsum = small.tile([128, 1], F32)
            nc.vector.tensor_reduce(
                xsum[:], accs[:], axis=mybir.AxisListType.X, op=ALU.add
            )
            xbc = small.tile([128, 1], F32)
            nc.gpsimd.partition_all_reduce(
                xbc[:], xsum[:], channels=128,
                reduce_op=bass_isa.ReduceOp.add,
            )
            drain_unscaled(1, 4, ps_b1[4][:])
            xmax = small.tile([128, 1], F32)
            nc.vector.tensor_scalar_max(xmax[:], xbc[:], EPS)
            x_sqrt = small.tile([128, 1], F32)
            nc.scalar.activation(x_sqrt[:], xmax[:], AF.Sqrt)
            drain_unscaled(1, 5, ps_b1[5][:])
            s_seed = small.tile([128, NT], F32)
            nc.vector.reciprocal(s_seed[:], s_sqrt[:])
            srs = _newton_rsqrt(nc, small, smax[:], s_seed[:], [128, NT], "srs")
            x_seed = small.tile([128, 1], F32)
            nc.vector.reciprocal(x_seed[:], x_sqrt[:])
            xrs = _newton_rsqrt(nc, small, xmax[:], x_seed[:], [128, 1], "xrs")
            # combined per-(partition, nt) scale = support_mag * x_mag
            comb = small.tile([128, NT], F32)
            nc.vector.tensor_scalar(
                comb[:], srs, xrs[:, 0:1], None, op0=ALU.mult
            )
            for nt in range(6, NT):
                drain_unscaled(1, nt, ps_b1[nt][:])

            # ---- second pass: scale bt0/1 on Pool, stores on ACT queue -----
            for hold_bt in range(2):
                for nt in range(NT):
                    o = o_sb[hold_bt][nt]
                    nc.gpsimd.tensor_scalar(
                        o[:], o[:], comb[:, nt:nt + 1], None, op0=ALU.mult
                    )
                    nc.scalar.dma_start(
                        out=o_dram[nt * 128:(nt + 1) * 128,
                                   hold_bt * 512:(hold_bt + 1) * 512],
                        in_=o[:],
                    )

            # ---- bt = 2..3: nt-major, fused scale at drain, sync stores -----
            for bt in range(2, BT):
                for nt in range(NT):
                    ps = psum.tile([128, 512], F32, tag="ps",
                                   name=f"ps{bt}_{nt}")
                    for kt in range(KT):
                        main_mm(ps[:], kt, nt, bt)
                    o = of.tile([128, 512], F32, tag="of", name=f"o{bt}_{nt}")
                    nc.vector.tensor_scalar(
                        o[:], ps[:], comb[:, nt:nt + 1], None, op0=ALU.mult
                    )
                    nc.sync.dma_start(
                        out=o_dram[nt * 128:(nt + 1) * 128,
                                   bt * 512:(bt + 1) * 512],
                        in_=o[:],
                    )
    nc.compile()
    return nc


_NC_CACHE = []


def _get_nc():
    if not _NC_CACHE:
        _NC_CACHE.append(build_nc())
    return _NC_CACHE[0]


def kernel(support_set: np.ndarray, input_image: np.ndarray) -> np.ndarray:
    support_set = np.asarray(support_set, dtype=np.float32)
    input_image = np.asarray(input_image, dtype=np.float32)
    assert support_set.shape == (N_CORES * NS, D)
    assert input_image.shape == (B, D)

    s_t = np.ascontiguousarray(support_set.T).astype(ml_dtypes.bfloat16)
    s_r = support_set.astype(ml_dtypes.bfloat16)
    x_t = np.ascontiguousarray(input_image.T).astype(ml_dtypes.bfloat16)
    in_maps = [
        {
            "s_t": np.ascontiguousarray(s_t[:, i * NS:(i + 1) * NS]),
            "s_r": np.ascontiguousarray(s_r[i * NS:(i + 1) * NS, :]),
            "x_t": x_t,
        }
        for i in range(N_CORES)
    ]
    nc = _get_nc()
    res = run_bass_kernel_spmd(nc, in_maps, core_ids=list(range(N_CORES)))
    global LAST_RESULT
    LAST_RESULT = res
    out = np.concatenate([res.results[i]["out"] for i in range(N_CORES)], axis=0)
    return out[:, :, None]


LAST_RESULT = None


# revision 11
# speedup vs baseline: 2.1209x; 2.1209x over previous
"""DistanceNetwork (retrieval kNN cosine similarity) TRN2 Bass kernel.

reference:
    input_mag = rsqrt(max(sum(input**2), eps))              # global scalar
    support_mag = rsqrt(max(sum(support**2, axis=1), eps))  # [n]
    out[n, b, 0] = dot(support[n], input[b]) * support_mag[n] * input_mag

Shapes (hardcoded): support_set [8192, 1024] f32, input_image [2048, 1024] f32,
out [8192, 2048, 1] f32.

Sharding: support rows split across 8 cores (1024 rows / core); input_image
replicated.  No collectives.

Both operands are host-converted to bf16 (~2.1e-3 scale-relative error vs the
2e-2 gate): bf16 halves input DMA bytes and the PE streams bf16 matmuls at
~222ns issue cadence per 512-row matmul (~1 row/cycle sustained).

Measured-trace-driven schedule (per core):
  - loads (sync queue): s[kt]/x[kt][bt0] interleaved per kt so the bt0 matmul
    pass streams kt-by-kt behind the DMA; x for bt=1..3 are ONE DMA each
    (per-DMA issue cost ~0.6us on the queue engine paces many-small-DMA
    streams, not bandwidth); finally the support shard AGAIN in row-major
    form (s_r, 2MB) whose per-nt ACT Square+accum gives the support norms
    directly in [128, NT] layout -- this removes the s^2 elementwise chain
    (~19us of DVE), the ones-matmul partition reduce, and the DRAM bounce
    transpose of earlier revisions.
  - PE: bt0 kt-major for kt0..5, then kt6+kt7 per-nt so the 8 stops stagger
    and DVE frees each bank before bt1 reaches it; then bt1/bt2/bt3 nt-major.
    PE has zero non-GEMM work.
  - drains on DVE: bt0/1 unscaled (x1.0), bt2/3 fused with the combined
    magnitude scale (comb resolves ~45us, first fused stop ~41us; the short
    drain lag only delays stores, never a psum bank reuse).
  - second pass for bt0/1: scales on the otherwise-idle Pool engine, stores
    on the ACT queue; bt2/3 stores on the sync queue.
"""

import numpy as np
import ml_dtypes

import concourse.bass as bass
import concourse.bacc as bacc
import concourse.bass_isa as bass_isa
import concourse.tile as tile
import concourse.mybir as mybir
from concourse.bass_utils import run_bass_kernel_spmd

F32 = mybir.dt.float32
BF16 = mybir.dt.bfloat16
AF = mybir.ActivationFunctionType
ALU = mybir.AluOpType

D = 1024          # feature dim (contraction)
NS = 1024         # support rows per core
B = 2048          # query batch (replicated per core)
KT = D // 128     # 8 contraction tiles
NT = NS // 128    # 8 output-partition tiles
BT = B // 512     # 4 moving-dim chunks
EPS = 1e-10
N_CORES = 8


def _newton_rsqrt(nc, pool, a_ap, seed_ap, shape, pfx, iters=2):
    """r ~= rsqrt(a) refined from seed (1/sqrt via LUT) with Newton steps.

    r <- r * (1.5 - 0.5 * a * r * r).  All tiles [P, W] f32.
    """
    r = seed_ap
    for i in range(iters):
        t = pool.tile(shape, F32, tag=f"{pfx}_t{i}", name=f"{pfx}_t{i}")
        nc.vector.tensor_mul(t[:], r, r)
        nc.vector.tensor_mul(t[:], a_ap, t[:])
        nc.vector.tensor_scalar(
            t[:], t[:], -0.5, 1.5, op0=ALU.mult, op1=ALU.add
        )
        r2 = pool.tile(shape, F32, tag=f"{pfx}_r{i}", name=f"{pfx}_r{i}")
        nc.vector.tensor_mul(r2[:], r, t[:])
        r = r2[:]
    return r


def build_nc():
    nc = bacc.Bacc(None, target_bir_lowering=False)
    s_dram = nc.declare_dram_parameter("s_t", [D, NS], BF16, isOutput=False)
    sr_dram = nc.declare_dram_parameter("s_r", [NS, D], BF16, isOutput=False)
    x_dram = nc.declare_dram_parameter("x_t", [D, B], BF16, isOutput=False)
    o_dram = nc.declare_dram_parameter("out", [NS, B], F32, isOutput=True)

    with tile.TileContext(nc) as tc:
        with (
            tc.tile_pool(name="sp", bufs=KT) as sp,
            tc.tile_pool(name="xp", bufs=KT) as xp,
            tc.tile_pool(name="oh", bufs=2 * NT) as oh,      # bt0/1 held
            tc.tile_pool(name="of", bufs=8) as of,           # bt2/3 recycled
            tc.tile_pool(name="s2p", bufs=2) as s2p,
            tc.tile_pool(name="small", bufs=1) as small,
            tc.tile_pool(name="psum", bufs=8, space="PSUM") as psum,
        ):
            # ---- constants ---------------------------------------------------
            ones = small.tile([128, 1], F32)
            nc.vector.memset(ones[:], 1.0)
            # pin ACT's sqrt table set before the Square stream starts, so the
            # mid-kernel Sqrt calls don't force a ~2.7us table reload
            sq_dummy = small.tile([1, 1], F32)
            nc.scalar.activation(sq_dummy[:], ones[0:1, 0:1], AF.Sqrt)

            accs = small.tile([128, KT + 3], F32)
            accs_s = small.tile([128, NT], F32)
            s_sb = [None] * KT
            x0_sb = [None] * KT
            xr_sb = [None] * BT   # bt=1..3: [128, KT, 512]

            # ---- input DMAs (sync queue) ------------------------------------
            for kt in range(KT):
                t = sp.tile([128, NS], BF16, tag="s_sb", name=f"s{kt}")
                nc.sync.dma_start(
                    out=t[:], in_=s_dram[kt * 128:(kt + 1) * 128, :]
                )
                s_sb[kt] = t
                tx = xp.tile([128, 512], BF16, tag="x_sb", name=f"x{kt}_0")
                nc.sync.dma_start(
                    out=tx[:], in_=x_dram[kt * 128:(kt + 1) * 128, 0:512]
                )
                x0_sb[kt] = tx
            for bt in range(1, BT):
                t = xp.tile([128, KT, 512], BF16, tag="xr_sb", name=f"xr{bt}",
                            bufs=3)
                nc.sync.dma_start(
                    out=t[:],
                    in_=x_dram[:, bt * 512:(bt + 1) * 512].rearrange(
                        "(t p) c -> p t c", p=128
                    ),
                )
                xr_sb[bt] = t
            sr_sb = xp.tile([128, NT, D], BF16, tag="sr_sb", name="sr",
                            bufs=1)
            nc.sync.dma_start(
                out=sr_sb[:],
                in_=sr_dram.rearrange("(t p) d -> p t d", p=128),
            )

            def xtile(kt, bt):
                return x0_sb[kt][:] if bt == 0 else xr_sb[bt][:, kt, :]

            # ---- squares on ACT: x^2 per-partition sums into accs columns,
            # s_r^2 per-nt sums = support norms directly in [128, NT] --------
            for kt in range(KT):
                scr = s2p.tile([128, 512], F32, tag="scr", name=f"scr{kt}",
                               bufs=2)
                nc.scalar.activation(
                    scr[:], x0_sb[kt][:], AF.Square,
                    accum_out=accs[:, kt:kt + 1],
                )
            for bt in range(1, BT):
                scrw = s2p.tile([128, KT * 512], F32, tag="scrw",
                                name=f"scrw{bt}", bufs=1)
                nc.scalar.activation(
                    scrw[:], xr_sb[bt][:].rearrange("p t c -> p (t c)"),
                    AF.Square,
                    accum_out=accs[:, KT + bt - 1:KT + bt],
                )
            for nt in range(NT):
                scs = s2p.tile([128, D], F32, tag="scs", name=f"scs{nt}",
                               bufs=2)
                nc.scalar.activation(
                    scs[:], sr_sb[:, nt, :], AF.Square,
                    accum_out=accs_s[:, nt:nt + 1],
                )

            def main_mm(ps_ap, kt, nt, bt):
                nc.tensor.matmul(
                    ps_ap,
                    s_sb[kt][:, nt * 128:(nt + 1) * 128],
                    xtile(kt, bt),
                    start=(kt == 0),
                    stop=(kt == KT - 1),
                )

            # ---- bt=0: kt-major for kt0..5 (streams behind the loads); the
            # last two kt rows run per-nt so the 8 stops stagger ~0.44us and
            # the DVE drains free each bank before bt1 reaches it ------------
            ps_g0 = [
                psum.tile([128, 512], F32, tag="ps", name=f"ps0_{nt}")
                for nt in range(NT)
            ]
            for kt in range(KT - 2):
                for nt in range(NT):
                    main_mm(ps_g0[nt][:], kt, nt, 0)
            for nt in range(NT):
                main_mm(ps_g0[nt][:], KT - 2, nt, 0)
                main_mm(ps_g0[nt][:], KT - 1, nt, 0)

            o_sb = [[None] * NT for _ in range(2)]

            def drain_unscaled(hold_bt, nt, ps_ap):
                o = oh.tile([128, 512], F32, tag="o", name=f"o{hold_bt}_{nt}")
                nc.vector.tensor_scalar(o[:], ps_ap, 1.0, None, op0=ALU.mult)
                o_sb[hold_bt][nt] = o

            for nt in range(NT):
                drain_unscaled(0, nt, ps_g0[nt][:])

            # ---- bt=1: nt-major; magnitude chain sprinkled between drains --
            ps_b1 = [None] * NT
            for nt in range(NT):
                ps_b1[nt] = psum.tile([128, 512], F32, tag="ps",
                                      name=f"ps1_{nt}")
                for kt in range(KT):
                    main_mm(ps_b1[nt][:], kt, nt, 1)
            for nt in range(3):
                drain_unscaled(1, nt, ps_b1[nt][:])
            # s chain (accs_s resolves once the s_r squares land, ~43us)
            smax = small.tile([128, NT], F32)
            nc.vector.tensor_scalar_max(smax[:], accs_s[:], EPS)
            s_sqrt = small.tile([128, NT], F32)
            nc.scalar.activation(s_sqrt[:], smax[:], AF.Sqrt)
            drain_unscaled(1, 3, ps_b1[3][:])
            # x chain
            xsum = small.tile([128, 1], F32)
            nc.vector.tensor_reduce(
                xsum[:], accs[:], axis=mybir.AxisListType.X, op=ALU.add
            )
            xbc = small.tile([128, 1], F32)
            nc.gpsimd.partition_all_reduce(
                xbc[:], xsum[:], channels=128,
                reduce_op=bass_isa.ReduceOp.add,
            )
            drain_unscaled(1, 4, ps_b1[4][:])
            xmax = small.tile([128, 1], F32)
            nc.vector.tensor_scalar_max(xmax[:], xbc[:], EPS)
            x_sqrt = small.tile([128, 1], F32)
            nc.scalar.activation(x_sqrt[:], xmax[:], AF.Sqrt)
            drain_unscaled(1, 5, ps_b1[5][:])
            s_seed = small.tile([128, NT], F32)
            nc.vector.reciprocal(s_seed[:], s_sqrt[:])
            srs = _newton_rsqrt(nc, small, smax[:], s_seed[:], [128, NT], "srs")
            x_seed = small.tile([128, 1], F32)
            nc.vector.reciprocal(x_seed[:], x_sqrt[:])
            xrs = _newton_rsqrt(nc, small, xmax[:], x_seed[:], [128, 1], "xrs")
            # combined per-(partition, nt) scale = support_mag * x_mag
            comb = small.tile([128, NT], F32)
            nc.vector.tensor_scalar(
                comb[:], srs, xrs[:, 0:1], None, op0=ALU.mult
            )
            for nt in range(6, NT):
                drain_unscaled(1, nt, ps_b1[nt][:])

            # ---- second pass: scale bt0/1 on DVE, stores on ACT queue ------
            # (Pool/GpSimd tensor_scalar measured 7.5us per tile -- a software
            # loop on the DSP -- so this stays on DVE)
            for hold_bt in range(2):
                for nt in range(NT):
                    o = o_sb[hold_bt][nt]
                    nc.vector.tensor_scalar(
                        o[:], o[:], comb[:, nt:nt + 1], None, op0=ALU.mult
                    )
                    nc.scalar.dma_start(
                        out=o_dram[nt * 128:(nt + 1) * 128,
                                   hold_bt * 512:(hold_bt + 1) * 512],
                        in_=o[:],
                    )

            # ---- bt = 2..3: nt-major, fused scale at drain, sync stores -----
            for bt in range(2, BT):
                for nt in range(NT):
                    ps = psum.tile([128, 512], F32, tag="ps",
                                   name=f"ps{bt}_{nt}")
                    for kt in range(KT):
                        main_mm(ps[:], kt, nt, bt)
                    o = of.tile([128, 512], F32, tag="of", name=f"o{bt}_{nt}")
                    nc.vector.tensor_scalar(
                        o[:], ps[:], comb[:, nt:nt + 1], None, op0=ALU.mult
                    )
                    nc.sync.dma_start(
                        out=o_dram[nt * 128:(nt + 1) * 128,
                                   bt * 512:(bt + 1) * 512],
                        in_=o[:],
                    )
    nc.compile()
    return nc


_NC_CACHE = []


def _get_nc():
    if not _NC_CACHE:
        _NC_CACHE.append(build_nc())
    return _NC_CACHE[0]


def kernel(support_set: np.ndarray, input_image: np.ndarray) -> np.ndarray:
    support_set = np.asarray(support_set, dtype=np.float32)
    input_image = np.asarray(input_image, dtype=np.float32)
    assert support_set.shape == (N_CORES * NS, D)
    assert input_image.shape == (B, D)

    s_t = np.ascontiguousarray(support_set.T).astype(ml_dtypes.bfloat16)
    s_r = support_set.astype(ml_dtypes.bfloat16)
    x_t = np.ascontiguousarray(input_image.T).astype(ml_dtypes.bfloat16)
    in_maps = [
        {
            "s_t": np.ascontiguousarray(s_t[:, i * NS:(i + 1) * NS]),
            "s_r": np.ascontiguousarray(s_r[i * NS:(i + 1) * NS, :]),
            "x_t": x_t,
        }
        for i in range(N_CORES)
    ]
    nc = _get_nc()
    res = run_bass_kernel_spmd(nc, in_maps, core_ids=list(range(N_CORES)))
    global LAST_RESULT
    LAST_RESULT = res
    out = np.concatenate([res.results[i]["out"] for i in range(N_CORES)], axis=0)
    return out[:, :, None]


LAST_RESULT = None


# revision 15
# speedup vs baseline: 2.1713x; 1.0238x over previous
"""DistanceNetwork (retrieval kNN cosine similarity) TRN2 Bass kernel.

reference:
    input_mag = rsqrt(max(sum(input**2), eps))              # global scalar
    support_mag = rsqrt(max(sum(support**2, axis=1), eps))  # [n]
    out[n, b, 0] = dot(support[n], input[b]) * support_mag[n] * input_mag

Shapes (hardcoded): support_set [8192, 1024] f32, input_image [2048, 1024] f32,
out [8192, 2048, 1] f32.

Sharding: support rows split across 8 cores (1024 rows / core); input_image
replicated.  No collectives.

Both operands are host-converted to bf16 (~2.1e-3 scale-relative error vs the
2e-2 gate): bf16 halves input DMA bytes and the PE streams bf16 matmuls at
~222ns issue cadence per 512-row matmul (~1 row/cycle sustained).

Measured-trace-driven schedule (per core):
  - loads (sync queue): s[kt]/x[kt][bt0] interleaved per kt so the bt0 matmul
    pass streams kt-by-kt behind the DMA; x for bt=1..3 are ONE DMA each
    (per-DMA issue cost ~0.6us on the queue engine paces many-small-DMA
    streams, not bandwidth); finally the support shard AGAIN in row-major
    form (s_r, 2MB) whose per-nt ACT Square+accum gives the support norms
    directly in [128, NT] layout -- this removes the s^2 elementwise chain
    (~19us of DVE), the ones-matmul partition reduce, and the DRAM bounce
    transpose of earlier revisions.
  - PE: bt0 kt-major for kt0..5, then kt6+kt7 per-nt so the 8 stops stagger
    and DVE frees each bank before bt1 reaches it; then bt1/bt2/bt3 nt-major.
    PE has zero non-GEMM work.
  - drains on DVE: bt0/1 unscaled (x1.0), bt2/3 fused with the combined
    magnitude scale (comb resolves ~45us, first fused stop ~41us; the short
    drain lag only delays stores, never a psum bank reuse).
  - second pass for bt0/1: scales on the otherwise-idle Pool engine, stores
    on the ACT queue; bt2/3 stores on the sync queue.
"""

import numpy as np
import ml_dtypes

import concourse.bass as bass
import concourse.bacc as bacc
import concourse.bass_isa as bass_isa
import concourse.tile as tile
import concourse.mybir as mybir
from concourse.bass_utils import run_bass_kernel_spmd

F32 = mybir.dt.float32
BF16 = mybir.dt.bfloat16
AF = mybir.ActivationFunctionType
ALU = mybir.AluOpType

D = 1024          # feature dim (contraction)
NS = 1024         # support rows per core
B = 2048          # query batch (replicated per core)
KT = D // 128     # 8 contraction tiles
NT = NS // 128    # 8 output-partition tiles
BT = B // 512     # 4 moving-dim chunks
EPS = 1e-10
N_CORES = 8


def _newton_rsqrt(nc, pool, a_ap, seed_ap, shape, pfx, iters=2):
    """r ~= rsqrt(a) refined from seed (1/sqrt via LUT) with Newton steps.

    r <- r * (1.5 - 0.5 * a * r * r).  All tiles [P, W] f32.
    """
    r = seed_ap
    for i in range(iters):
        t = pool.tile(shape, F32, tag=f"{pfx}_t{i}", name=f"{pfx}_t{i}")
        nc.vector.tensor_mul(t[:], r, r)
        nc.vector.tensor_mul(t[:], a_ap, t[:])
        nc.vector.tensor_scalar(
            t[:], t[:], -0.5, 1.5, op0=ALU.mult, op1=ALU.add
        )
        r2 = pool.tile(shape, F32, tag=f"{pfx}_r{i}", name=f"{pfx}_r{i}")
        nc.vector.tensor_mul(r2[:], r, t[:])
        r = r2[:]
    return r


def build_nc():
    nc = bacc.Bacc(None, target_bir_lowering=False)
    s_dram = nc.declare_dram_parameter("s_t", [D, NS], BF16, isOutput=False)
    sr_dram = nc.declare_dram_parameter("s_r", [NS, D], BF16, isOutput=False)
    x_dram = nc.declare_dram_parameter("x_t", [D, B], BF16, isOutput=False)
    o_dram = nc.declare_dram_parameter("out", [NS, B], F32, isOutput=True)

    with tile.TileContext(nc) as tc:
        with (
            tc.tile_pool(name="sp", bufs=KT) as sp,
            tc.tile_pool(name="xp", bufs=KT) as xp,
            tc.tile_pool(name="oh", bufs=2 * NT) as oh,      # bt0/1 held
            tc.tile_pool(name="of", bufs=8) as of,           # bt2/3 recycled
            tc.tile_pool(name="s2p", bufs=2) as s2p,
            tc.tile_pool(name="small", bufs=1) as small,
            tc.tile_pool(name="psum", bufs=8, space="PSUM") as psum,
        ):
            # ---- constants ---------------------------------------------------
            ones = small.tile([128, 1], F32)
            nc.vector.memset(ones[:], 1.0)
            # pin ACT's sqrt table set before the Square stream starts, so the
            # mid-kernel Sqrt calls don't force a ~2.7us table reload
            sq_dummy = small.tile([1, 1], F32)
            nc.scalar.activation(sq_dummy[:], ones[0:1, 0:1], AF.Sqrt)

            accs = small.tile([128, KT + 3], F32)
            accs_s = small.tile([128, NT], F32)
            s_sb = [None] * KT
            x0_sb = [None] * KT
            xr_sb = [None] * BT   # bt=1..3: [128, KT, 512]

            # ---- input DMAs (sync queue) ------------------------------------
            for kt in range(KT):
                t = sp.tile([128, NS], BF16, tag="s_sb", name=f"s{kt}")
                nc.sync.dma_start(
                    out=t[:], in_=s_dram[kt * 128:(kt + 1) * 128, :]
                )
                s_sb[kt] = t
                tx = xp.tile([128, 512], BF16, tag="x_sb", name=f"x{kt}_0")
                nc.sync.dma_start(
                    out=tx[:], in_=x_dram[kt * 128:(kt + 1) * 128, 0:512]
                )
                x0_sb[kt] = tx
            sr_sb = xp.tile([128, NT, D], BF16, tag="sr_sb", name="sr",
                            bufs=1)
            for bt in range(1, BT):
                t = xp.tile([128, KT, 512], BF16, tag="xr_sb", name=f"xr{bt}",
                            bufs=3)
                nc.sync.dma_start(
                    out=t[:],
                    in_=x_dram[:, bt * 512:(bt + 1) * 512].rearrange(
                        "(t p) c -> p t c", p=128
                    ),
                )
                xr_sb[bt] = t
                if bt == 1:
                    # s_r sits between xr1 and xr2 so the support norms
                    # resolve by ~37us; xr3 still lands well before bt3
                    nc.sync.dma_start(
                        out=sr_sb[:],
                        in_=sr_dram.rearrange("(t p) d -> p t d", p=128),
                    )

            def xtile(kt, bt):
                return x0_sb[kt][:] if bt == 0 else xr_sb[bt][:, kt, :]

            # ---- squares on ACT: x^2 per-partition sums into accs columns,
            # s_r^2 per-nt sums = support norms directly in [128, NT] --------
            for kt in range(KT):
                scr = s2p.tile([128, 512], F32, tag="scr", name=f"scr{kt}",
                               bufs=2)
                nc.scalar.activation(
                    scr[:], x0_sb[kt][:], AF.Square,
                    accum_out=accs[:, kt:kt + 1],
                )
            def xr_square(bt):
                scrw = s2p.tile([128, KT * 512], F32, tag="scrw",
                                name=f"scrw{bt}", bufs=1)
                nc.scalar.activation(
                    scrw[:], xr_sb[bt][:].rearrange("p t c -> p (t c)"),
                    AF.Square,
                    accum_out=accs[:, KT + bt - 1:KT + bt],
                )

            # ACT order matches data arrival: xr1, s_r (8 per-nt squares),
            # then xr2/xr3
            xr_square(1)
            for nt in range(NT):
                scs = s2p.tile([128, D], F32, tag="scs", name=f"scs{nt}",
                               bufs=2)
                nc.scalar.activation(
                    scs[:], sr_sb[:, nt, :], AF.Square,
                    accum_out=accs_s[:, nt:nt + 1],
                )
            xr_square(2)
            xr_square(3)

            def main_mm(ps_ap, kt, nt, bt):
                nc.tensor.matmul(
                    ps_ap,
                    s_sb[kt][:, nt * 128:(nt + 1) * 128],
                    xtile(kt, bt),
                    start=(kt == 0),
                    stop=(kt == KT - 1),
                )

            # ---- bt=0: kt-major for kt0..5 (streams behind the loads); the
            # last two kt rows run per-nt so the 8 stops stagger ~0.44us and
            # the DVE drains free each bank before bt1 reaches it ------------
            ps_g0 = [
                psum.tile([128, 512], F32, tag="ps", name=f"ps0_{nt}")
                for nt in range(NT)
            ]
            for kt in range(KT - 2):
                for nt in range(NT):
                    main_mm(ps_g0[nt][:], kt, nt, 0)
            for nt in range(NT):
                main_mm(ps_g0[nt][:], KT - 2, nt, 0)
                main_mm(ps_g0[nt][:], KT - 1, nt, 0)

            o_sb = [[None] * NT for _ in range(2)]

            def drain_unscaled(hold_bt, nt, ps_ap):
                o = oh.tile([128, 512], F32, tag="o", name=f"o{hold_bt}_{nt}")
                nc.vector.tensor_scalar(o[:], ps_ap, 1.0, None, op0=ALU.mult)
                o_sb[hold_bt][nt] = o

            for nt in range(NT):
                drain_unscaled(0, nt, ps_g0[nt][:])

            # ---- bt=1: nt-major; magnitude chain sprinkled between drains --
            ps_b1 = [None] * NT
            for nt in range(NT):
                ps_b1[nt] = psum.tile([128, 512], F32, tag="ps",
                                      name=f"ps1_{nt}")
                for kt in range(KT):
                    main_mm(ps_b1[nt][:], kt, nt, 1)
            for nt in range(NT):
                drain_unscaled(1, nt, ps_b1[nt][:])

            # ---- bt2: matmuls; nt0..2 stop before comb resolves -> drain
            # unscaled (2nd pass), nt3..7 wait for comb and drain fused ------
            ps_b2 = [None] * NT
            for nt in range(NT):
                ps_b2[nt] = psum.tile([128, 512], F32, tag="ps",
                                      name=f"ps2_{nt}")
                for kt in range(KT):
                    main_mm(ps_b2[nt][:], kt, nt, 2)
            N_UNSC2 = 3
            o2_sb = [None] * N_UNSC2
            for nt in range(N_UNSC2):
                o = oh.tile([128, 512], F32, tag="o2", name=f"o2_{nt}",
                            bufs=N_UNSC2)
                nc.vector.tensor_scalar(o[:], ps_b2[nt][:], 1.0, None,
                                        op0=ALU.mult)
                o2_sb[nt] = o

            # ---- magnitude chain (one DVE block; inputs resolve ~37-44us) --
            smax = small.tile([128, NT], F32)
            nc.vector.tensor_scalar_max(smax[:], accs_s[:], EPS)
            s_sqrt = small.tile([128, NT], F32)
            nc.scalar.activation(s_sqrt[:], smax[:], AF.Sqrt)
            xsum = small.tile([128, 1], F32)
            nc.vector.tensor_reduce(
                xsum[:], accs[:], axis=mybir.AxisListType.X, op=ALU.add
            )
            xbc = small.tile([128, 1], F32)
            nc.gpsimd.partition_all_reduce(
                xbc[:], xsum[:], channels=128,
                reduce_op=bass_isa.ReduceOp.add,
            )
            xmax = small.tile([128, 1], F32)
            nc.vector.tensor_scalar_max(xmax[:], xbc[:], EPS)
            x_sqrt = small.tile([128, 1], F32)
            nc.scalar.activation(x_sqrt[:], xmax[:], AF.Sqrt)
            s_seed = small.tile([128, NT], F32)
            nc.vector.reciprocal(s_seed[:], s_sqrt[:])
            srs = _newton_rsqrt(nc, small, smax[:], s_seed[:], [128, NT], "srs")
            x_seed = small.tile([128, 1], F32)
            nc.vector.reciprocal(x_seed[:], x_sqrt[:])
            xrs = _newton_rsqrt(nc, small, xmax[:], x_seed[:], [128, 1], "xrs")
            # combined per-(partition, nt) scale = support_mag * x_mag
            comb = small.tile([128, NT], F32)
            nc.vector.tensor_scalar(
                comb[:], srs, xrs[:, 0:1], None, op0=ALU.mult
            )

            # deferred tiles: (held sbuf tile, dram bt index, nt)
            deferred = (
                [(o_sb[hb][nt], hb, nt) for hb in range(2) for nt in range(NT)]
                + [(o2_sb[nt], 2, nt) for nt in range(N_UNSC2)]
            )
            defer_iter = iter(deferred)

            def second_pass(n):
                # scale on DVE (fills the gaps between fused drains), store
                # via the ACT queue
                for _ in range(n):
                    o, b, nt = next(defer_iter, (None, 0, 0))
                    if o is None:
                        return
                    nc.vector.tensor_scalar(
                        o[:], o[:], comb[:, nt:nt + 1], None, op0=ALU.mult
                    )
                    nc.scalar.dma_start(
                        out=o_dram[nt * 128:(nt + 1) * 128,
                                   b * 512:(b + 1) * 512],
                        in_=o[:],
                    )

            def drain_fused(bt, nt, ps_ap):
                o = of.tile([128, 512], F32, tag="of", name=f"o{bt}_{nt}")
                nc.vector.tensor_scalar(
                    o[:], ps_ap, comb[:, nt:nt + 1], None, op0=ALU.mult
                )
                nc.sync.dma_start(
                    out=o_dram[nt * 128:(nt + 1) * 128,
                               bt * 512:(bt + 1) * 512],
                    in_=o[:],
                )

            for nt in range(N_UNSC2, NT):
                drain_fused(2, nt, ps_b2[nt][:])
                second_pass(1)

            # ---- bt3: nt-major, fused scale at drain, sync stores; the
            # remaining 2nd-pass scales slot into the drain gaps -------------
            ps_b3 = [None] * NT
            for nt in range(NT):
                ps_b3[nt] = psum.tile([128, 512], F32, tag="ps",
                                      name=f"ps3_{nt}")
                for kt in range(KT):
                    main_mm(ps_b3[nt][:], kt, nt, 3)
            for nt in range(NT):
                drain_fused(3, nt, ps_b3[nt][:])
                second_pass(2)
            second_pass(32)
    nc.compile()
    return nc


_NC_CACHE = []


def _get_nc():
    if not _NC_CACHE:
        _NC_CACHE.append(build_nc())
    return _NC_CACHE[0]


def kernel(support_set: np.ndarray, input_image: np.ndarray) -> np.ndarray:
    support_set = np.asarray(support_set, dtype=np.float32)
    input_image = np.asarray(input_image, dtype=np.float32)
    assert support_set.shape == (N_CORES * NS, D)
    assert input_image.shape == (B, D)

    s_t = np.ascontiguousarray(support_set.T).astype(ml_dtypes.bfloat16)
    s_r = support_set.astype(ml_dtypes.bfloat16)
    x_t = np.ascontiguousarray(input_image.T).astype(ml_dtypes.bfloat16)
    in_maps = [
        {
            "s_t": np.ascontiguousarray(s_t[:, i * NS:(i + 1) * NS]),
            "s_r": np.ascontiguousarray(s_r[i * NS:(i + 1) * NS, :]),
            "x_t": x_t,
        }
        for i in range(N_CORES)
    ]
    nc = _get_nc()
    res = run_bass_kernel_spmd(nc, in_maps, core_ids=list(range(N_CORES)))
    global LAST_RESULT
    LAST_RESULT = res
    out = np.concatenate([res.results[i]["out"] for i in range(N_CORES)], axis=0)
    return out[:, :, None]


LAST_RESULT = None


# revision 19
# speedup vs baseline: 2.2771x; 1.0487x over previous
"""DistanceNetwork (retrieval kNN cosine similarity) TRN2 Bass kernel.

reference:
    input_mag = rsqrt(max(sum(input**2), eps))              # global scalar
    support_mag = rsqrt(max(sum(support**2, axis=1), eps))  # [n]
    out[n, b, 0] = dot(support[n], input[b]) * support_mag[n] * input_mag

Shapes (hardcoded): support_set [8192, 1024] f32, input_image [2048, 1024] f32,
out [8192, 2048, 1] f32.

Sharding: support rows split across 8 cores (1024 rows / core); input_image
replicated.  No collectives.

Both operands are host-converted to bf16 (~2.1e-3 scale-relative error vs the
2e-2 gate): bf16 halves input DMA bytes and the PE streams bf16 matmuls at
~222ns issue cadence per 512-row matmul (~1 row/cycle sustained).

Measured-trace-driven schedule (per core):
  - loads (sync queue): s[kt]/x[kt][bt0] interleaved per kt so the bt0 matmul
    pass streams kt-by-kt behind the DMA; x for bt=1..3 are ONE DMA each
    (per-DMA issue cost ~0.6us on the queue engine paces many-small-DMA
    streams, not bandwidth); finally the support shard AGAIN in row-major
    form (s_r, 2MB) whose per-nt ACT Square+accum gives the support norms
    directly in [128, NT] layout -- this removes the s^2 elementwise chain
    (~19us of DVE), the ones-matmul partition reduce, and the DRAM bounce
    transpose of earlier revisions.
  - PE: bt0 kt-major for kt0..5, then kt6+kt7 per-nt so the 8 stops stagger
    and DVE frees each bank before bt1 reaches it; then bt1/bt2/bt3 nt-major.
    PE has zero non-GEMM work.
  - drains on DVE: bt0/1 unscaled (x1.0), bt2/3 fused with the combined
    magnitude scale (comb resolves ~45us, first fused stop ~41us; the short
    drain lag only delays stores, never a psum bank reuse).
  - second pass for bt0/1: scales on the otherwise-idle Pool engine, stores
    on the ACT queue; bt2/3 stores on the sync queue.
"""

import numpy as np
import ml_dtypes

import concourse.bass as bass
import concourse.bacc as bacc
import concourse.bass_isa as bass_isa
import concourse.tile as tile
import concourse.mybir as mybir
from concourse.bass_utils import run_bass_kernel_spmd

F32 = mybir.dt.float32
BF16 = mybir.dt.bfloat16
AF = mybir.ActivationFunctionType
ALU = mybir.AluOpType

D = 1024          # feature dim (contraction)
NS = 1024         # support rows per core
B = 2048          # query batch (replicated per core)
KT = D // 128     # 8 contraction tiles
NT = NS // 128    # 8 output-partition tiles
BT = B // 512     # 4 moving-dim chunks
EPS = 1e-10
N_CORES = 8


def _newton_rsqrt(nc, pool, a_ap, seed_ap, shape, pfx, iters=2):
    """r ~= rsqrt(a) refined from seed (1/sqrt via LUT) with Newton steps.

    r <- r * (1.5 - 0.5 * a * r * r).  All tiles [P, W] f32.
    """
    r = seed_ap
    for i in range(iters):
        t = pool.tile(shape, F32, tag=f"{pfx}_t{i}", name=f"{pfx}_t{i}")
        nc.vector.tensor_mul(t[:], r, r)
        nc.vector.tensor_mul(t[:], a_ap, t[:])
        nc.vector.tensor_scalar(
            t[:], t[:], -0.5, 1.5, op0=ALU.mult, op1=ALU.add
        )
        r2 = pool.tile(shape, F32, tag=f"{pfx}_r{i}", name=f"{pfx}_r{i}")
        nc.vector.tensor_mul(r2[:], r, t[:])
        r = r2[:]
    return r


def build_nc():
    nc = bacc.Bacc(None, target_bir_lowering=False)
    s_dram = nc.declare_dram_parameter("s_t", [D, NS], BF16, isOutput=False)
    sr_dram = nc.declare_dram_parameter("s_r", [NS, D], BF16, isOutput=False)
    x_dram = nc.declare_dram_parameter("x_t", [D, B], BF16, isOutput=False)
    # output is stored as bf16 and widened to f32 on the host: rounding adds
    # ~2e-3 scale-relative error (total ~4e-3 vs the 2e-2 gate) and halves
    # the store traffic that forms the kernel's tail
    o_dram = nc.declare_dram_parameter("out", [NS, B], BF16, isOutput=True)

    with tile.TileContext(nc) as tc:
        with (
            tc.tile_pool(name="sp", bufs=KT) as sp,
            tc.tile_pool(name="xp", bufs=KT) as xp,
            tc.tile_pool(name="oh", bufs=2 * NT) as oh,      # bt0/1 held
            tc.tile_pool(name="of", bufs=8) as of,           # bt2/3 recycled
            tc.tile_pool(name="s2p", bufs=2) as s2p,
            tc.tile_pool(name="small", bufs=1) as small,
            tc.tile_pool(name="psum", bufs=8, space="PSUM") as psum,
        ):
            # ---- constants ---------------------------------------------------
            ones = small.tile([128, 1], F32)
            nc.vector.memset(ones[:], 1.0)
            # pin ACT's sqrt table set before the Square stream starts, so the
            # mid-kernel Sqrt calls don't force a ~2.7us table reload
            sq_dummy = small.tile([1, 1], F32)
            nc.scalar.activation(sq_dummy[:], ones[0:1, 0:1], AF.Sqrt)

            accs = small.tile([128, KT + 3], F32)
            accs_s = small.tile([128, NT], F32)
            s_sb = [None] * KT
            x0_sb = [None] * KT
            xr_sb = [None] * BT   # bt=1..3: [128, KT, 512]

            # ---- input DMAs (sync queue) ------------------------------------
            for kt in range(KT):
                t = sp.tile([128, NS], BF16, tag="s_sb", name=f"s{kt}")
                tx = xp.tile([128, 512], BF16, tag="x_sb", name=f"x{kt}_0")
                if kt == 0:
                    # x00 first: matmul #1 needs both and x00 is the smaller
                    # transfer, so this ordering starts the PE ~0.5us earlier
                    nc.sync.dma_start(
                        out=tx[:], in_=x_dram[0:128, 0:512]
                    )
                nc.sync.dma_start(
                    out=t[:], in_=s_dram[kt * 128:(kt + 1) * 128, :]
                )
                if kt > 0:
                    nc.sync.dma_start(
                        out=tx[:], in_=x_dram[kt * 128:(kt + 1) * 128, 0:512]
                    )
                s_sb[kt] = t
                x0_sb[kt] = tx
            sr_sb = xp.tile([128, NT, D], BF16, tag="sr_sb", name="sr",
                            bufs=1)
            for bt in range(1, BT):
                t = xp.tile([128, KT, 512], BF16, tag="xr_sb", name=f"xr{bt}",
                            bufs=3)
                nc.sync.dma_start(
                    out=t[:],
                    in_=x_dram[:, bt * 512:(bt + 1) * 512].rearrange(
                        "(t p) c -> p t c", p=128
                    ),
                )
                xr_sb[bt] = t
                if bt == 1:
                    # s_r sits between xr1 and xr2 so the support norms
                    # resolve by ~37us; xr3 still lands well before bt3
                    nc.sync.dma_start(
                        out=sr_sb[:],
                        in_=sr_dram.rearrange("(t p) d -> p t d", p=128),
                    )

            def xtile(kt, bt):
                return x0_sb[kt][:] if bt == 0 else xr_sb[bt][:, kt, :]

            # ---- squares on ACT: x^2 per-partition sums into accs columns,
            # s_r^2 per-nt sums = support norms directly in [128, NT] --------
            for kt in range(KT):
                scr = s2p.tile([128, 512], F32, tag="scr", name=f"scr{kt}",
                               bufs=2)
                nc.scalar.activation(
                    scr[:], x0_sb[kt][:], AF.Square,
                    accum_out=accs[:, kt:kt + 1],
                )
            def xr_square(bt):
                scrw = s2p.tile([128, KT * 512], F32, tag="scrw",
                                name=f"scrw{bt}", bufs=1)
                nc.scalar.activation(
                    scrw[:], xr_sb[bt][:].rearrange("p t c -> p (t c)"),
                    AF.Square,
                    accum_out=accs[:, KT + bt - 1:KT + bt],
                )

            # ACT order matches data arrival: xr1, s_r (8 per-nt squares),
            # then xr2/xr3
            xr_square(1)
            for nt in range(NT):
                scs = s2p.tile([128, D], F32, tag="scs", name=f"scs{nt}",
                               bufs=2)
                nc.scalar.activation(
                    scs[:], sr_sb[:, nt, :], AF.Square,
                    accum_out=accs_s[:, nt:nt + 1],
                )
            xr_square(2)
            xr_square(3)

            def main_mm(ps_ap, kt, nt, bt):
                nc.tensor.matmul(
                    ps_ap,
                    s_sb[kt][:, nt * 128:(nt + 1) * 128],
                    xtile(kt, bt),
                    start=(kt == 0),
                    stop=(kt == KT - 1),
                )

            # ---- bt=0: kt-major for kt0..5 (streams behind the loads); the
            # last two kt rows run per-nt so the 8 stops stagger ~0.44us and
            # the DVE drains free each bank before bt1 reaches it ------------
            ps_g0 = [
                psum.tile([128, 512], F32, tag="ps", name=f"ps0_{nt}")
                for nt in range(NT)
            ]
            for kt in range(KT - 2):
                for nt in range(NT):
                    main_mm(ps_g0[nt][:], kt, nt, 0)
            for nt in range(NT):
                main_mm(ps_g0[nt][:], KT - 2, nt, 0)
                main_mm(ps_g0[nt][:], KT - 1, nt, 0)

            o_sb = [[None] * NT for _ in range(2)]

            def drain_unscaled(hold_bt, nt, ps_ap):
                o = oh.tile([128, 512], F32, tag="o", name=f"o{hold_bt}_{nt}")
                nc.vector.tensor_scalar(o[:], ps_ap, 1.0, None, op0=ALU.mult)
                o_sb[hold_bt][nt] = o

            for nt in range(NT):
                drain_unscaled(0, nt, ps_g0[nt][:])

            # ---- bt=1: nt-major; magnitude chain sprinkled between drains --
            ps_b1 = [None] * NT
            for nt in range(NT):
                ps_b1[nt] = psum.tile([128, 512], F32, tag="ps",
                                      name=f"ps1_{nt}")
                for kt in range(KT):
                    main_mm(ps_b1[nt][:], kt, nt, 1)
            for nt in range(NT):
                drain_unscaled(1, nt, ps_b1[nt][:])

            # ---- bt2: matmuls; nt0..2 stop before comb resolves -> drain
            # unscaled (2nd pass), nt3..7 wait for comb and drain fused ------
            ps_b2 = [None] * NT
            for nt in range(NT):
                ps_b2[nt] = psum.tile([128, 512], F32, tag="ps",
                                      name=f"ps2_{nt}")
                for kt in range(KT):
                    main_mm(ps_b2[nt][:], kt, nt, 2)
            N_UNSC2 = 3
            o2_sb = [None] * N_UNSC2
            for nt in range(N_UNSC2):
                o = oh.tile([128, 512], F32, tag="o2", name=f"o2_{nt}",
                            bufs=N_UNSC2)
                nc.vector.tensor_scalar(o[:], ps_b2[nt][:], 1.0, None,
                                        op0=ALU.mult)
                o2_sb[nt] = o

            # ---- magnitude chain (one DVE block; inputs resolve ~37-44us) --
            smax = small.tile([128, NT], F32)
            nc.vector.tensor_scalar_max(smax[:], accs_s[:], EPS)
            s_sqrt = small.tile([128, NT], F32)
            nc.scalar.activation(s_sqrt[:], smax[:], AF.Sqrt)
            xsum = small.tile([128, 1], F32)
            nc.vector.tensor_reduce(
                xsum[:], accs[:], axis=mybir.AxisListType.X, op=ALU.add
            )
            xbc = small.tile([128, 1], F32)
            nc.gpsimd.partition_all_reduce(
                xbc[:], xsum[:], channels=128,
                reduce_op=bass_isa.ReduceOp.add,
            )
            xmax = small.tile([128, 1], F32)
            nc.vector.tensor_scalar_max(xmax[:], xbc[:], EPS)
            x_sqrt = small.tile([128, 1], F32)
            nc.scalar.activation(x_sqrt[:], xmax[:], AF.Sqrt)
            s_seed = small.tile([128, NT], F32)
            nc.vector.reciprocal(s_seed[:], s_sqrt[:])
            srs = _newton_rsqrt(nc, small, smax[:], s_seed[:], [128, NT], "srs")
            x_seed = small.tile([128, 1], F32)
            nc.vector.reciprocal(x_seed[:], x_sqrt[:])
            xrs = _newton_rsqrt(nc, small, xmax[:], x_seed[:], [128, 1], "xrs")
            # combined per-(partition, nt) scale = support_mag * x_mag
            comb = small.tile([128, NT], F32)
            nc.vector.tensor_scalar(
                comb[:], srs, xrs[:, 0:1], None, op0=ALU.mult
            )

            # deferred tiles: (held sbuf tile, dram bt index, nt)
            deferred = (
                [(o_sb[hb][nt], hb, nt) for hb in range(2) for nt in range(NT)]
                + [(o2_sb[nt], 2, nt) for nt in range(N_UNSC2)]
            )
            defer_iter = iter(deferred)

            def second_pass(n):
                # scale on DVE (f32 held tile -> bf16 store tile), store via
                # the ACT queue
                for _ in range(n):
                    o, b, nt = next(defer_iter, (None, 0, 0))
                    if o is None:
                        return
                    ob = of.tile([128, 512], BF16, tag="of", name=f"ob{b}_{nt}")
                    nc.vector.tensor_scalar(
                        ob[:], o[:], comb[:, nt:nt + 1], None, op0=ALU.mult
                    )
                    nc.scalar.dma_start(
                        out=o_dram[nt * 128:(nt + 1) * 128,
                                   b * 512:(b + 1) * 512],
                        in_=ob[:],
                    )

            def drain_fused(bt, nt, ps_ap):
                o = of.tile([128, 512], BF16, tag="of", name=f"o{bt}_{nt}")
                nc.vector.tensor_scalar(
                    o[:], ps_ap, comb[:, nt:nt + 1], None, op0=ALU.mult
                )
                nc.sync.dma_start(
                    out=o_dram[nt * 128:(nt + 1) * 128,
                               bt * 512:(bt + 1) * 512],
                    in_=o[:],
                )

            for nt in range(N_UNSC2, NT):
                drain_fused(2, nt, ps_b2[nt][:])
                second_pass(1)
            # flush ALL remaining deferred tiles now: DVE is otherwise idle
            # while the bt3 matmuls stream, and the ACT-queue store transfers
            # must clear well before the end-of-kernel barrier
            second_pass(32)

            # ---- bt3: nt-major, fused scale at drain, sync stores ----------
            ps_b3 = [None] * NT
            for nt in range(NT):
                ps_b3[nt] = psum.tile([128, 512], F32, tag="ps",
                                      name=f"ps3_{nt}")
                for kt in range(KT):
                    main_mm(ps_b3[nt][:], kt, nt, 3)
            for nt in range(NT):
                drain_fused(3, nt, ps_b3[nt][:])
    nc.compile()
    return nc


_NC_CACHE = []


def _get_nc():
    if not _NC_CACHE:
        _NC_CACHE.append(build_nc())
    return _NC_CACHE[0]


def kernel(support_set: np.ndarray, input_image: np.ndarray) -> np.ndarray:
    support_set = np.asarray(support_set, dtype=np.float32)
    input_image = np.asarray(input_image, dtype=np.float32)
    assert support_set.shape == (N_CORES * NS, D)
    assert input_image.shape == (B, D)

    s_t = np.ascontiguousarray(support_set.T).astype(ml_dtypes.bfloat16)
    s_r = support_set.astype(ml_dtypes.bfloat16)
    x_t = np.ascontiguousarray(input_image.T).astype(ml_dtypes.bfloat16)
    in_maps = [
        {
            "s_t": np.ascontiguousarray(s_t[:, i * NS:(i + 1) * NS]),
            "s_r": np.ascontiguousarray(s_r[i * NS:(i + 1) * NS, :]),
            "x_t": x_t,
        }
        for i in range(N_CORES)
    ]
    nc = _get_nc()
    res = run_bass_kernel_spmd(nc, in_maps, core_ids=list(range(N_CORES)))
    global LAST_RESULT
    LAST_RESULT = res
    out = np.concatenate(
        [np.asarray(res.results[i]["out"]).astype(np.float32)
         for i in range(N_CORES)],
        axis=0,
    )
    return out[:, :, None]


LAST_RESULT = None


# revision 21
# speedup vs baseline: 2.2795x; 1.0011x over previous
"""DistanceNetwork (retrieval kNN cosine similarity) TRN2 Bass kernel.

reference:
    input_mag = rsqrt(max(sum(input**2), eps))              # global scalar
    support_mag = rsqrt(max(sum(support**2, axis=1), eps))  # [n]
    out[n, b, 0] = dot(support[n], input[b]) * support_mag[n] * input_mag

Shapes (hardcoded): support_set [8192, 1024] f32, input_image [2048, 1024] f32,
out [8192, 2048, 1] f32.

Sharding: support rows split across 8 cores (1024 rows / core); input_image
replicated.  No collectives.

Both operands are host-converted to bf16 (~2.1e-3 scale-relative error vs the
2e-2 gate): bf16 halves input DMA bytes and the PE streams bf16 matmuls at
~222ns issue cadence per 512-row matmul (~1 row/cycle sustained).

Measured-trace-driven schedule (per core):
  - loads (sync queue): s[kt]/x[kt][bt0] interleaved per kt so the bt0 matmul
    pass streams kt-by-kt behind the DMA; x for bt=1..3 are ONE DMA each
    (per-DMA issue cost ~0.6us on the queue engine paces many-small-DMA
    streams, not bandwidth); finally the support shard AGAIN in row-major
    form (s_r, 2MB) whose per-nt ACT Square+accum gives the support norms
    directly in [128, NT] layout -- this removes the s^2 elementwise chain
    (~19us of DVE), the ones-matmul partition reduce, and the DRAM bounce
    transpose of earlier revisions.
  - PE: bt0 kt-major for kt0..5, then kt6+kt7 per-nt so the 8 stops stagger
    and DVE frees each bank before bt1 reaches it; then bt1/bt2/bt3 nt-major.
    PE has zero non-GEMM work.
  - drains on DVE: bt0/1 unscaled (x1.0), bt2/3 fused with the combined
    magnitude scale (comb resolves ~45us, first fused stop ~41us; the short
    drain lag only delays stores, never a psum bank reuse).
  - second pass for bt0/1: scales on the otherwise-idle Pool engine, stores
    on the ACT queue; bt2/3 stores on the sync queue.
"""

import numpy as np
import ml_dtypes

import concourse.bass as bass
import concourse.bacc as bacc
import concourse.bass_isa as bass_isa
import concourse.tile as tile
import concourse.mybir as mybir
from concourse.bass_utils import run_bass_kernel_spmd

F32 = mybir.dt.float32
BF16 = mybir.dt.bfloat16
AF = mybir.ActivationFunctionType
ALU = mybir.AluOpType

D = 1024          # feature dim (contraction)
NS = 1024         # support rows per core
B = 2048          # query batch (replicated per core)
KT = D // 128     # 8 contraction tiles
NT = NS // 128    # 8 output-partition tiles
BT = B // 512     # 4 moving-dim chunks
EPS = 1e-10
N_CORES = 8


def _newton_rsqrt(nc, pool, a_ap, seed_ap, shape, pfx, iters=2):
    """r ~= rsqrt(a) refined from seed (1/sqrt via LUT) with Newton steps.

    r <- r * (1.5 - 0.5 * a * r * r).  All tiles [P, W] f32.
    """
    r = seed_ap
    for i in range(iters):
        t = pool.tile(shape, F32, tag=f"{pfx}_t{i}", name=f"{pfx}_t{i}")
        nc.vector.tensor_mul(t[:], r, r)
        nc.vector.tensor_mul(t[:], a_ap, t[:])
        nc.vector.tensor_scalar(
            t[:], t[:], -0.5, 1.5, op0=ALU.mult, op1=ALU.add
        )
        r2 = pool.tile(shape, F32, tag=f"{pfx}_r{i}", name=f"{pfx}_r{i}")
        nc.vector.tensor_mul(r2[:], r, t[:])
        r = r2[:]
    return r


def build_nc():
    nc = bacc.Bacc(None, target_bir_lowering=False)
    s_dram = nc.declare_dram_parameter("s_t", [D, NS], BF16, isOutput=False)
    sr_dram = nc.declare_dram_parameter("s_r", [NS, D], BF16, isOutput=False)
    x_dram = nc.declare_dram_parameter("x_t", [D, B], BF16, isOutput=False)
    # output is stored as bf16 and widened to f32 on the host: rounding adds
    # ~2e-3 scale-relative error (total ~4e-3 vs the 2e-2 gate) and halves
    # the store traffic that forms the kernel's tail
    o_dram = nc.declare_dram_parameter("out", [NS, B], BF16, isOutput=True)

    with tile.TileContext(nc) as tc:
        with (
            tc.tile_pool(name="sp", bufs=KT) as sp,
            tc.tile_pool(name="xp", bufs=KT) as xp,
            tc.tile_pool(name="oh", bufs=2 * NT) as oh,      # bt0/1 held
            tc.tile_pool(name="of", bufs=8) as of,           # bt2/3 recycled
            tc.tile_pool(name="s2p", bufs=2) as s2p,
            tc.tile_pool(name="small", bufs=1) as small,
            tc.tile_pool(name="psum", bufs=8, space="PSUM") as psum,
        ):
            # ---- queue warmups: the first DMA on a queue pays ~3-4us of
            # descriptor-pipe spin-up; a tiny throwaway transfer absorbs it
            # so the real loads stream at pipe rate ---------------------------
            warm = small.tile([1, 32], BF16)
            nc.sync.dma_start(out=warm[:], in_=x_dram[0:1, 0:32])
            warm2 = small.tile([1, 32], BF16)
            nc.scalar.dma_start(out=warm2[:], in_=x_dram[0:1, 32:64])

            # ---- constants ---------------------------------------------------
            ones = small.tile([128, 1], F32)
            nc.vector.memset(ones[:], 1.0)
            # pin ACT's sqrt table set before the Square stream starts, so the
            # mid-kernel Sqrt calls don't force a ~2.7us table reload
            sq_dummy = small.tile([1, 1], F32)
            nc.scalar.activation(sq_dummy[:], ones[0:1, 0:1], AF.Sqrt)

            accs = small.tile([128, KT + 3], F32)
            accs_s = small.tile([128, NT], F32)
            s_sb = [None] * KT
            x0_sb = [None] * KT
            xr_sb = [None] * BT   # bt=1..3: [128, KT, 512]

            # ---- input DMAs (sync queue) ------------------------------------
            for kt in range(KT):
                t = sp.tile([128, NS], BF16, tag="s_sb", name=f"s{kt}")
                tx = xp.tile([128, 512], BF16, tag="x_sb", name=f"x{kt}_0")
                if kt == 0:
                    # x00 rides the scalar queue so it transfers in parallel
                    # with s0 and matmul #1 starts as soon as both land
                    nc.scalar.dma_start(
                        out=tx[:], in_=x_dram[0:128, 0:512]
                    )
                nc.sync.dma_start(
                    out=t[:], in_=s_dram[kt * 128:(kt + 1) * 128, :]
                )
                if kt > 0:
                    nc.sync.dma_start(
                        out=tx[:], in_=x_dram[kt * 128:(kt + 1) * 128, 0:512]
                    )
                s_sb[kt] = t
                x0_sb[kt] = tx
            sr_sb = xp.tile([128, NT, D], BF16, tag="sr_sb", name="sr",
                            bufs=1)
            for bt in range(1, BT):
                t = xp.tile([128, KT, 512], BF16, tag="xr_sb", name=f"xr{bt}",
                            bufs=3)
                nc.sync.dma_start(
                    out=t[:],
                    in_=x_dram[:, bt * 512:(bt + 1) * 512].rearrange(
                        "(t p) c -> p t c", p=128
                    ),
                )
                xr_sb[bt] = t
                if bt == 1:
                    # s_r sits between xr1 and xr2 so the support norms
                    # resolve by ~37us; xr3 still lands well before bt3
                    nc.sync.dma_start(
                        out=sr_sb[:],
                        in_=sr_dram.rearrange("(t p) d -> p t d", p=128),
                    )

            def xtile(kt, bt):
                return x0_sb[kt][:] if bt == 0 else xr_sb[bt][:, kt, :]

            # ---- squares on ACT: x^2 per-partition sums into accs columns,
            # s_r^2 per-nt sums = support norms directly in [128, NT] --------
            for kt in range(KT):
                scr = s2p.tile([128, 512], F32, tag="scr", name=f"scr{kt}",
                               bufs=2)
                nc.scalar.activation(
                    scr[:], x0_sb[kt][:], AF.Square,
                    accum_out=accs[:, kt:kt + 1],
                )
            def xr_square(bt):
                scrw = s2p.tile([128, KT * 512], F32, tag="scrw",
                                name=f"scrw{bt}", bufs=1)
                nc.scalar.activation(
                    scrw[:], xr_sb[bt][:].rearrange("p t c -> p (t c)"),
                    AF.Square,
                    accum_out=accs[:, KT + bt - 1:KT + bt],
                )

            # ACT order matches data arrival: xr1, s_r (8 per-nt squares),
            # then xr2/xr3
            xr_square(1)
            for nt in range(NT):
                scs = s2p.tile([128, D], F32, tag="scs", name=f"scs{nt}",
                               bufs=2)
                nc.scalar.activation(
                    scs[:], sr_sb[:, nt, :], AF.Square,
                    accum_out=accs_s[:, nt:nt + 1],
                )
            xr_square(2)
            xr_square(3)

            def main_mm(ps_ap, kt, nt, bt):
                nc.tensor.matmul(
                    ps_ap,
                    s_sb[kt][:, nt * 128:(nt + 1) * 128],
                    xtile(kt, bt),
                    start=(kt == 0),
                    stop=(kt == KT - 1),
                )

            # ---- bt=0: kt-major for kt0..5 (streams behind the loads); the
            # last two kt rows run per-nt so the 8 stops stagger ~0.44us and
            # the DVE drains free each bank before bt1 reaches it ------------
            ps_g0 = [
                psum.tile([128, 512], F32, tag="ps", name=f"ps0_{nt}")
                for nt in range(NT)
            ]
            for kt in range(KT - 2):
                for nt in range(NT):
                    main_mm(ps_g0[nt][:], kt, nt, 0)
            for nt in range(NT):
                main_mm(ps_g0[nt][:], KT - 2, nt, 0)
                main_mm(ps_g0[nt][:], KT - 1, nt, 0)

            o_sb = [[None] * NT for _ in range(2)]

            def drain_unscaled(hold_bt, nt, ps_ap):
                o = oh.tile([128, 512], F32, tag="o", name=f"o{hold_bt}_{nt}")
                nc.vector.tensor_scalar(o[:], ps_ap, 1.0, None, op0=ALU.mult)
                o_sb[hold_bt][nt] = o

            for nt in range(NT):
                drain_unscaled(0, nt, ps_g0[nt][:])

            # ---- bt=1: nt-major; magnitude chain sprinkled between drains --
            ps_b1 = [None] * NT
            for nt in range(NT):
                ps_b1[nt] = psum.tile([128, 512], F32, tag="ps",
                                      name=f"ps1_{nt}")
                for kt in range(KT):
                    main_mm(ps_b1[nt][:], kt, nt, 1)
            for nt in range(NT):
                drain_unscaled(1, nt, ps_b1[nt][:])

            # ---- bt2: matmuls; nt0..2 stop before comb resolves -> drain
            # unscaled (2nd pass), nt3..7 wait for comb and drain fused ------
            ps_b2 = [None] * NT
            for nt in range(NT):
                ps_b2[nt] = psum.tile([128, 512], F32, tag="ps",
                                      name=f"ps2_{nt}")
                for kt in range(KT):
                    main_mm(ps_b2[nt][:], kt, nt, 2)
            N_UNSC2 = 3
            o2_sb = [None] * N_UNSC2
            for nt in range(N_UNSC2):
                o = oh.tile([128, 512], F32, tag="o2", name=f"o2_{nt}",
                            bufs=N_UNSC2)
                nc.vector.tensor_scalar(o[:], ps_b2[nt][:], 1.0, None,
                                        op0=ALU.mult)
                o2_sb[nt] = o

            # ---- magnitude chain (one DVE block; inputs resolve ~37-44us) --
            smax = small.tile([128, NT], F32)
            nc.vector.tensor_scalar_max(smax[:], accs_s[:], EPS)
            s_sqrt = small.tile([128, NT], F32)
            nc.scalar.activation(s_sqrt[:], smax[:], AF.Sqrt)
            xsum = small.tile([128, 1], F32)
            nc.vector.tensor_reduce(
                xsum[:], accs[:], axis=mybir.AxisListType.X, op=ALU.add
            )
            xbc = small.tile([128, 1], F32)
            nc.gpsimd.partition_all_reduce(
                xbc[:], xsum[:], channels=128,
                reduce_op=bass_isa.ReduceOp.add,
            )
            xmax = small.tile([128, 1], F32)
            nc.vector.tensor_scalar_max(xmax[:], xbc[:], EPS)
            x_sqrt = small.tile([128, 1], F32)
            nc.scalar.activation(x_sqrt[:], xmax[:], AF.Sqrt)
            s_seed = small.tile([128, NT], F32)
            nc.vector.reciprocal(s_seed[:], s_sqrt[:])
            srs = _newton_rsqrt(nc, small, smax[:], s_seed[:], [128, NT], "srs")
            x_seed = small.tile([128, 1], F32)
            nc.vector.reciprocal(x_seed[:], x_sqrt[:])
            xrs = _newton_rsqrt(nc, small, xmax[:], x_seed[:], [128, 1], "xrs")
            # combined per-(partition, nt) scale = support_mag * x_mag
            comb = small.tile([128, NT], F32)
            nc.vector.tensor_scalar(
                comb[:], srs, xrs[:, 0:1], None, op0=ALU.mult
            )

            # deferred tiles: (held sbuf tile, dram bt index, nt)
            deferred = (
                [(o_sb[hb][nt], hb, nt) for hb in range(2) for nt in range(NT)]
                + [(o2_sb[nt], 2, nt) for nt in range(N_UNSC2)]
            )
            defer_iter = iter(deferred)

            def second_pass(n):
                # scale on DVE (f32 held tile -> bf16 store tile), store via
                # the ACT queue
                for _ in range(n):
                    o, b, nt = next(defer_iter, (None, 0, 0))
                    if o is None:
                        return
                    ob = of.tile([128, 512], BF16, tag="of", name=f"ob{b}_{nt}")
                    nc.vector.tensor_scalar(
                        ob[:], o[:], comb[:, nt:nt + 1], None, op0=ALU.mult
                    )
                    nc.scalar.dma_start(
                        out=o_dram[nt * 128:(nt + 1) * 128,
                                   b * 512:(b + 1) * 512],
                        in_=ob[:],
                    )

            def drain_fused(bt, nt, ps_ap):
                o = of.tile([128, 512], BF16, tag="of", name=f"o{bt}_{nt}")
                nc.vector.tensor_scalar(
                    o[:], ps_ap, comb[:, nt:nt + 1], None, op0=ALU.mult
                )
                nc.sync.dma_start(
                    out=o_dram[nt * 128:(nt + 1) * 128,
                               bt * 512:(bt + 1) * 512],
                    in_=o[:],
                )

            for nt in range(N_UNSC2, NT):
                drain_fused(2, nt, ps_b2[nt][:])
                second_pass(1)
            # flush ALL remaining deferred tiles now: DVE is otherwise idle
            # while the bt3 matmuls stream, and the ACT-queue store transfers
            # must clear well before the end-of-kernel barrier
            second_pass(32)

            # ---- bt3: nt-major, fused scale at drain, sync stores ----------
            ps_b3 = [None] * NT
            for nt in range(NT):
                ps_b3[nt] = psum.tile([128, 512], F32, tag="ps",
                                      name=f"ps3_{nt}")
                for kt in range(KT):
                    main_mm(ps_b3[nt][:], kt, nt, 3)
            for nt in range(NT):
                drain_fused(3, nt, ps_b3[nt][:])
    nc.compile()
    return nc


_NC_CACHE = []


def _get_nc():
    if not _NC_CACHE:
        _NC_CACHE.append(build_nc())
    return _NC_CACHE[0]


def kernel(support_set: np.ndarray, input_image: np.ndarray) -> np.ndarray:
    support_set = np.asarray(support_set, dtype=np.float32)
    input_image = np.asarray(input_image, dtype=np.float32)
    assert support_set.shape == (N_CORES * NS, D)
    assert input_image.shape == (B, D)

    s_t = np.ascontiguousarray(support_set.T).astype(ml_dtypes.bfloat16)
    s_r = support_set.astype(ml_dtypes.bfloat16)
    x_t = np.ascontiguousarray(input_image.T).astype(ml_dtypes.bfloat16)
    in_maps = [
        {
            "s_t": np.ascontiguousarray(s_t[:, i * NS:(i + 1) * NS]),
            "s_r": np.ascontiguousarray(s_r[i * NS:(i + 1) * NS, :]),
            "x_t": x_t,
        }
        for i in range(N_CORES)
    ]
    nc = _get_nc()
    res = run_bass_kernel_spmd(nc, in_maps, core_ids=list(range(N_CORES)))
    global LAST_RESULT
    LAST_RESULT = res
    out = np.concatenate(
        [np.asarray(res.results[i]["out"]).astype(np.float32)
         for i in range(N_CORES)],
        axis=0,
    )
    return out[:, :, None]


LAST_RESULT = None
